# revision 22
# baseline (speedup 1.0000x reference)
"""Trainium2 Bass kernel for BondEmbedding (GNN edge embedding).

out[e, :] = concat(bond_feat[e], gaussian_smearing(|pos[i0[e]] - pos[i1[e]]|)) @ W + b

Sharding: edges split across 8 NeuronCores (embarrassingly parallel);
pos table / weights / constants replicated on every core.

Per-core dataflow (supertile = 4096 edges, K=32 edges per SBUF partition,
edge e0 + p*K + kk lives at slot (p, kk)):
  - HWDGE DMA: bond_feat slab, block-idx / remainder slabs, output stores
  - SWDGE dma_gather (gpsimd queues 0/1, issued one supertile ahead): the
    pos table is packed [25000, 64] f32 (4 nodes per 256B block, each node
    a 16B row); per edge endpoint the 256B block is gathered (block index
    fits the required int16), then the node's 16B row is selected on DVE
    via a 4-wide one-hot and a grouped reduce.
  - DVE: diff/square/reduce -> dist2, clamp; d = dist2 * rsqrt(dist2)
    via the int bit-hack seed + 2 Newton iterations (keeps ACT's
    activation-table fixed on the exp set -- the table-load pass picks
    disjoint sets for Ln and Exp, costing 2x1283ns per tile otherwise);
    u = d - offsets, u^2
  - ACT: gauss = exp(coeff*u^2); psum -> sbuf bf16 copies of bond featT
  - PE (bf16 matmul = 1 cycle/row vs 4 for f32): bond features are
    transposed two 64-wide chunks at a time ([128,128] f32 -> [128,128]
    psum), gauss+ones four 21-wide chunks at a time ([128,84] f32 ->
    [84,128] psum); after bf16 copies to SBUF, each 128-edge chunk is
    two accumulating matmuls (K=64 bond + K=21 gauss) against the
    matching row-blocks of Wb [85,128] bf16. Row 84 of the gauss block
    is constant 1.0 and row 84 of Wb is the bias, so the bias needs no
    separate add.
  - Pool: indirect-gather dispatch; gauss psum -> sbuf bf16 copies
  - DVE: out psum -> sbuf copy, one 4-chunk store DMA per group

bf16 is used only for the matmul operands (bond features, gauss features,
weights); positions/distances stay f32. Max rel err vs the f32 reference
is ~1e-3, well inside the 2e-2 gate.
"""

import sys

sys.path.insert(0, "/opt/trn_rl_repo")

import numpy as np

E_TOTAL = 2_000_000
N_NODES = 100_000
IN_DIM = 64
OUT_DIM = 128
NG = 20
CUTOFF = 10.0
FEAT = IN_DIM + NG + 1  # 85 = 64 bond + 20 gauss + 1 ones (bias row)
GPAD = 32  # gauss slot stride (20 gauss + 1 one + 11 pad); 32 aligns packs to legal PE base partitions {0,32,64}

N_BLOCKS = N_NODES // 4  # 25000 blocks of 4 nodes (256B each)
N_CORES = 8
SHARD = E_TOTAL // N_CORES  # 250000
K = 32                      # edges per partition per supertile
S = 128 * K                 # 4096 edges per supertile
NT = 62                     # supertiles per core
E_PC = S * NT               # 253952 edges per core (wrap-around overlap)

_DELTA = CUTOFF / (NG - 1)
COEFF = -0.5 / (_DELTA * _DELTA)

_prog_cache = {}
WORK_BUFS = 4


def build_program(e_pc, nt, k, repeat=1):
    """Build the per-core Bass program (identical on all cores).

    repeat>1 re-runs the whole edge sweep that many times (same inputs and
    outputs) — used only for slope-based wall-clock timing.
    """
    from concourse import bacc, mybir, tile, bass
    from concourse.masks import make_identity

    f32 = mybir.dt.float32
    i16 = mybir.dt.int16
    bf16 = mybir.dt.bfloat16
    i32 = mybir.dt.int32
    ALU = mybir.AluOpType
    ACT = mybir.ActivationFunctionType

    s = 128 * k

    nc = bacc.Bacc("TRN2", target_bir_lowering=False, debug=False)

    bond = nc.dram_tensor("bond_feat", [e_pc, IN_DIM], f32, kind="ExternalInput")
    blk0 = nc.dram_tensor("blk0", [nt, 128, (128 * k) // 16], i16, kind="ExternalInput")
    blk1 = nc.dram_tensor("blk1", [nt, 128, (128 * k) // 16], i16, kind="ExternalInput")
    rem0 = nc.dram_tensor("rem0", [nt, 128, k], f32, kind="ExternalInput")
    rem1 = nc.dram_tensor("rem1", [nt, 128, k], f32, kind="ExternalInput")
    tab = nc.dram_tensor("tab", [N_BLOCKS, 64], f32, kind="ExternalInput")
    wb2 = nc.dram_tensor("wb2", [128, OUT_DIM], bf16, kind="ExternalInput")
    wg3 = nc.dram_tensor("wg3", [85, OUT_DIM], bf16, kind="ExternalInput")
    offs = nc.dram_tensor("offs", [128, NG], f32, kind="ExternalInput")
    out = nc.dram_tensor("out", [e_pc, OUT_DIM], f32, kind="ExternalOutput")

    with tile.TileContext(nc) as tc:
        with (
            tc.tile_pool(name="const", bufs=1) as cpool,
            tc.tile_pool(name="work", bufs=WORK_BUFS) as pool,
            tc.tile_pool(name="ft", bufs=8) as fpool,
            tc.tile_pool(name="gth", bufs=3) as gpool,
            tc.tile_pool(name="scratch", bufs=2) as spool,
            tc.tile_pool(name="psumb", bufs=3, space="PSUM") as ppoolb,
            tc.tile_pool(name="psumg", bufs=2, space="PSUM") as ppoolg,
            tc.tile_pool(name="psumo", bufs=3, space="PSUM") as ppoolo,
        ):
            wb2_sb = cpool.tile([128, OUT_DIM], bf16, tag="wb2")
            nc.sync.dma_start(out=wb2_sb[:], in_=wb2[:, :])
            wg3_sb = cpool.tile([85, OUT_DIM], bf16, tag="wg3")
            nc.sync.dma_start(out=wg3_sb[:], in_=wg3[:, :])
            offs_sb = cpool.tile([128, NG], f32, tag="offs")
            nc.sync.dma_start(out=offs_sb[:], in_=offs[:, :])
            ident = cpool.tile([128, 128], f32, tag="ident")
            make_identity(nc, ident[:])
            magic_sb = cpool.tile([128, 1], i32, tag="magic")
            nc.vector.memset(magic_sb[:], 0x5F3759DF)
            cand_sb = cpool.tile([128, 4], f32, tag="cand")
            nc.gpsimd.iota(cand_sb[:], pattern=[[1, 4]], base=0, channel_multiplier=0, allow_small_or_imprecise_dtypes=True)

            nw = s // 16

            def load_tiles(t):
                # issued two iterations ahead so the in-order SP queue
                # never parks loads behind store waits
                bt0 = pool.tile([128, nw], i16, tag="bt0")
                nc.sync.dma_start(out=bt0[:], in_=blk0[t, :, :])
                bt1 = pool.tile([128, nw], i16, tag="bt1")
                nc.sync.dma_start(out=bt1[:], in_=blk1[t, :, :])
                rt0 = pool.tile([128, k], f32, tag="rt0")
                nc.sync.dma_start(out=rt0[:], in_=rem0[t, :, :])
                rt1 = pool.tile([128, k], f32, tag="rt1")
                nc.sync.dma_start(out=rt1[:], in_=rem1[t, :, :])
                bf = pool.tile([128, IN_DIM * k], f32, tag="bf")
                nc.sync.dma_start(
                    out=bf[:],
                    in_=bond[t * s : t * s + s, :].rearrange(
                        "(p k) f -> p (k f)", p=128
                    ),
                )
                return (bt0, bt1, rt0, rt1), bf

            def gather(bts):
                # issued one iteration ahead: blocks are resident when the
                # iteration's DVE select/distance chain starts
                gths = []
                for r in range(2):
                    gth = gpool.tile([128, (s // 128) * 64], f32, tag=f"gth{r}")
                    nc.gpsimd.dma_gather(
                        out_ap=gth[:].rearrange("p (k c) -> p k c", c=64),
                        in_ap=tab[:, :],
                        idxs_ap=bts[r][:],
                        num_idxs=s,
                        num_idxs_reg=s,
                        elem_size=64,
                        single_packet=False,
                        queue_num=0,
                    )
                    gths.append(gth)
                return gths

            npack = (k + 2) // 3
            ngrp = (npack + 3) // 4

            cur = load_tiles(0)
            nxt = load_tiles(1 % nt)
            nxt2 = load_tiles(2 % nt)
            gth = gather(cur[0])
            gth_nxt = gather(nxt[0])
            for step in range(nt * repeat):
                t = step % nt
                e0 = t * s
                (bt0, bt1, rt0, rt1), bf = cur
                fut = load_tiles((t + 3) % nt)
                gth_nxt2 = gather(nxt2[0])
                cur, nxt, nxt2 = nxt, nxt2, fut

                # --- PE: bond transposes first (bf is already resident) --
                # 2 chunks per transpose ([128,128] f32); chunk kk lands at
                # partition base 64*(kk%2), col slot (kk//2)%4 of ptb kk//8.
                ptbs = []
                for j in range(k // 8):
                    ptb = ppoolb.tile([128, 4 * 128], f32, tag="ptb")
                    for i in range(4):
                        nc.tensor.transpose(
                            out=ptb[:, 128 * i : 128 * (i + 1)],
                            in_=bf[:, 128 * (4 * j + i) : 128 * (4 * j + i) + 128],
                            identity=ident[:],
                        )
                    ptbs.append(ptb)

                # --- DVE: select node rows, then distance chain ----------
                sel = []
                for r, rt in enumerate((rt0, rt1)):
                    oh = spool.tile([128, 4 * k], f32, tag=f"oh{r}")
                    nc.vector.tensor_tensor(
                        out=oh[:].rearrange("p (k m) -> p k m", m=4),
                        in0=rt[:].unsqueeze(2).to_broadcast([128, k, 4]),
                        in1=cand_sb[:].unsqueeze(1).to_broadcast([128, k, 4]),
                        op=ALU.is_equal,
                    )
                    tmp = spool.tile([128, 16 * k], f32, tag=f"tmp{r}")
                    gv = gth[r][:].rearrange("p (k m v) -> p k v m", m=4, v=16)
                    nc.vector.tensor_tensor(
                        out=tmp[:].rearrange("p (k c m) -> p k c m", c=4, m=4),
                        in0=gv[:, :, 0:4, :],
                        in1=oh[:]
                        .rearrange("p (k m) -> p k m", m=4)
                        .unsqueeze(2)
                        .to_broadcast([128, k, 4, 4]),
                        op=ALU.mult,
                    )
                    rr = spool.tile([128, 4 * k], f32, tag=f"r{r}")
                    nc.vector.tensor_reduce(
                        out=rr[:].rearrange("p (k c) -> p k c", c=4),
                        in_=tmp[:].rearrange("p (k c m) -> p k c m", c=4, m=4),
                        axis=mybir.AxisListType.X,
                        op=ALU.add,
                    )
                    sel.append(rr)
                diff = spool.tile([128, 4 * k], f32, tag="diff")
                diffv = diff[:].rearrange("p (k c) -> p k c", c=4)
                nc.vector.tensor_tensor(
                    out=diff[:], in0=sel[0][:], in1=sel[1][:], op=ALU.subtract
                )
                sq = spool.tile([128, 4 * k], f32, tag="sq")
                nc.vector.tensor_tensor(out=sq[:], in0=diff[:], in1=diff[:], op=ALU.mult)
                dist2 = spool.tile([128, k], f32, tag="dist2")
                nc.vector.tensor_reduce(
                    out=dist2[:],
                    in_=sq[:].rearrange("p (k c) -> p k c", c=4),
                    axis=mybir.AxisListType.X,
                    op=ALU.add,
                )
                # clamp keeps the Newton iterate finite (x*y^2 stays ~1)
                nc.vector.tensor_scalar_max(out=dist2[:], in0=dist2[:], scalar1=1e-30)
                ti = spool.tile([128, k], i32, tag="ti")
                nc.vector.tensor_scalar(
                    out=ti[:], in0=dist2[:].bitcast(i32), scalar1=1, scalar2=None,
                    op0=ALU.logical_shift_right,
                )
                y = spool.tile([128, k], f32, tag="y")
                nc.vector.tensor_tensor(
                    out=y[:].bitcast(i32),
                    in0=magic_sb[:].to_broadcast([128, k]),
                    in1=ti[:],
                    op=ALU.subtract,
                )
                nt1 = spool.tile([128, k], f32, tag="nt1")
                for _ in range(2):
                    nc.vector.tensor_tensor(out=nt1[:], in0=y[:], in1=y[:], op=ALU.mult)
                    # nt1 = (nt1 * -0.5) * dist2 ; y = (nt1 + 1.5) * y
                    nc.vector.scalar_tensor_tensor(
                        out=nt1[:], in0=nt1[:], scalar=-0.5, in1=dist2[:],
                        op0=ALU.mult, op1=ALU.mult,
                    )
                    nc.vector.scalar_tensor_tensor(
                        out=y[:], in0=nt1[:], scalar=1.5, in1=y[:],
                        op0=ALU.add, op1=ALU.mult,
                    )
                d = spool.tile([128, k], f32, tag="d")
                nc.vector.tensor_tensor(out=d[:], in0=dist2[:], in1=y[:], op=ALU.mult)

                # --- gauss u^2 in padded [128, k, GPAD] f32 tile ---------
                # slots NG:GPAD are zeroed: after the fused exp they become
                # 1.0 -- slot NG is the ones row that carries the bias.
                u = spool.tile([128, NG * k], f32, tag="u")
                uv = u[:].rearrange("p (k g) -> p k g", g=NG)
                nc.vector.tensor_tensor(
                    out=uv,
                    in0=d[:].unsqueeze(2).to_broadcast([128, k, NG]),
                    in1=offs_sb[:].unsqueeze(1).to_broadcast([128, k, NG]),
                    op=ALU.subtract,
                )
                uq = spool.tile([128, GPAD * k], f32, tag="uq")
                uqv = uq[:].rearrange("p (k g) -> p k g", g=GPAD)
                nc.vector.memset(uqv[:, :, NG:GPAD], 0.0)
                nc.vector.tensor_tensor(
                    out=uqv[:, :, 0:NG], in0=uv, in1=uv, op=ALU.mult
                )

                # --- PE: transpose u^2; ACT fuses exp into psum->sbuf ----
                # 3 chunks per transpose ([128,96] f32 incl pad); chunk kk
                # at base 32*(kk%3), col slot (kk//3)%4 of ftg (kk//12).
                gt3 = uq[:].rearrange("p (k g) -> p k g", g=GPAD)
                ftgs = []
                for g in range(ngrp):
                    pk0 = 4 * g
                    npk = min(4, npack - pk0)
                    ptg = ppoolg.tile([96, 4 * 128], f32, tag="ptg")
                    for pi in range(npk):
                        c0 = 3 * (pk0 + pi)
                        ncch = min(3, k - c0)
                        if ncch < 3:
                            nc.vector.memset(
                                ptg[32 * ncch : 96, 128 * pi : 128 * (pi + 1)], 0.0
                            )
                        nc.tensor.transpose(
                            out=ptg[0 : 32 * ncch, 128 * pi : 128 * (pi + 1)],
                            in_=gt3[:, c0 : c0 + ncch, :],
                            identity=ident[:],
                        )
                    ftg = fpool.tile([96, 4 * 128], bf16, tag="ftg")
                    nc.scalar.activation(
                        out=ftg[:, 0 : 128 * npk],
                        in_=ptg[:, 0 : 128 * npk],
                        func=ACT.Exp,
                        scale=COEFF,
                    )
                    ftgs.append(ftg)
                # DVE: bond psum -> sbuf bf16 (emitted after the distance
                # chain, so it never blocks the next iteration's DVE work)
                ftbs = []
                for j in range(k // 8):
                    ftb = fpool.tile([128, 4 * 128], bf16, tag="ftb")
                    nc.vector.tensor_copy(out=ftb[:], in_=ptbs[j][:])
                    ftbs.append(ftb)

                # --- matmuls + out copies + stores -----------------------
                ov = out[e0 : e0 + s, :].rearrange("(p kk) o -> p kk o", p=128)
                po = None
                for kk in range(k):
                    m = kk % 4
                    if m == 0:
                        po = ppoolo.tile([128, 4 * 128], f32, tag="po")
                    ftb = ftbs[kk // 8]
                    i = kk % 8
                    pk, gm = kk // 3, kk % 3
                    nc.tensor.matmul(
                        out=po[:, 128 * m : 128 * (m + 1)],
                        lhsT=ftb[
                            64 * (i % 2) : 64 * (i % 2) + 64,
                            128 * (i // 2) : 128 * (i // 2) + 128,
                        ],
                        rhs=wb2_sb[64 * (i % 2) : 64 * (i % 2) + 64, :],
                        start=True,
                        stop=False,
                    )
                    nc.tensor.matmul(
                        out=po[:, 128 * m : 128 * (m + 1)],
                        lhsT=ftgs[pk // 4][
                            32 * gm : 32 * gm + 21,
                            128 * (pk % 4) : 128 * (pk % 4) + 128,
                        ],
                        rhs=wg3_sb[32 * gm : 32 * gm + 21, :],
                        start=False,
                        stop=True,
                    )
                    if m == 3:
                        osb = fpool.tile([128, 4 * 128], f32, tag="osb")
                        nc.scalar.activation(out=osb[:], in_=po[:], func=ACT.Copy)
                        nc.sync.dma_start(
                            out=ov[:, kk - 3 : kk + 1, :],
                            in_=osb[:].rearrange("p (q o) -> p q o", o=OUT_DIM),
                        )

                gth, gth_nxt = gth_nxt, gth_nxt2

    nc.compile()
    return nc


def get_program(e_pc=E_PC, nt=NT, k=K):
    key = (e_pc, nt, k)
    if key not in _prog_cache:
        _prog_cache[key] = build_program(e_pc, nt, k)
    return _prog_cache[key]


def make_in_maps(bond_feat, bond_index, pos_nodes, W, b, e_pc=E_PC, nt=NT, k=K):
    """Shard the full problem into per-core input maps.

    Core c handles edges [c*SHARD, c*SHARD + e_pc) (wrapping around at
    E_TOTAL); rows beyond the first SHARD are redundant overlap so every
    core runs the identical static program.
    """
    import ml_dtypes

    bond_feat = np.ascontiguousarray(bond_feat, dtype=np.float32)
    idx0_all = np.ascontiguousarray(bond_index[0], dtype=np.int32)
    idx1_all = np.ascontiguousarray(bond_index[1], dtype=np.int32)

    tab = np.zeros((N_NODES, 16), dtype=np.float32)
    tab[:, :3] = pos_nodes
    tab = tab.reshape(N_BLOCKS, 64)

    offs_row = np.linspace(0.0, CUTOFF, NG, dtype=np.float32)
    offs_bcast = np.ascontiguousarray(np.broadcast_to(offs_row, (128, NG)))

    Wf = np.asarray(W, dtype=np.float32)
    wb2 = np.concatenate([Wf[:IN_DIM], Wf[:IN_DIM]], axis=0).astype(ml_dtypes.bfloat16)
    wg = np.concatenate(
        [Wf[IN_DIM:], np.asarray(b, dtype=np.float32)[None, :]], axis=0
    )  # [21, 128]
    wg3 = np.zeros((85, OUT_DIM), dtype=np.float32)
    for base in (0, 32, 64):
        wg3[base : base + 21] = wg
    wg3 = wg3.astype(ml_dtypes.bfloat16)

    def wrap_slice(arr, start, n):
        end = start + n
        if end <= E_TOTAL:
            return arr[start:end]
        return np.concatenate([arr[start:], arr[: end - E_TOTAL]], axis=0)

    in_maps = []
    for c in range(N_CORES):
        start = c * SHARD
        i0 = wrap_slice(idx0_all, start, e_pc)
        i1 = wrap_slice(idx1_all, start, e_pc)
        b0, r0 = _gather_inputs(i0, nt, k)
        b1, r1 = _gather_inputs(i1, nt, k)
        in_maps.append(
            {
                "bond_feat": wrap_slice(bond_feat, start, e_pc),
                "blk0": b0,
                "blk1": b1,
                "rem0": r0,
                "rem1": r1,
                "tab": tab,
                "wb2": wb2,
                "wg3": wg3,
                "offs": offs_bcast,
            }
        )
    return in_maps


def _gather_inputs(idx, nt, k):
    """blk (wrapped+replicated int16 block idx) and rem (f32 idx%4) slabs."""
    s = 128 * k
    nw = s // 16
    # gather-position i covers local edge (i%128)*k + i//128
    ii = np.arange(s)
    perm = (ii % 128) * k + (ii // 128)
    blk = (idx >> 2).astype(np.int16).reshape(nt, s)[:, perm]  # [nt, s]
    wrapped = blk.reshape(nt, nw, 16).transpose(0, 2, 1)  # [nt, 16, nw]
    blk_t = np.broadcast_to(wrapped[:, None, :, :], (nt, 8, 16, nw)).reshape(
        nt, 128, nw
    )
    rem = (idx & 3).astype(np.float32).reshape(nt, 128, k)
    return np.ascontiguousarray(blk_t), np.ascontiguousarray(rem)


def kernel(bond_feat, bond_index, pos_nodes, W, b):
    from concourse.bass_utils import run_bass_kernel_spmd

    nc = get_program()
    in_maps = make_in_maps(bond_feat, bond_index, pos_nodes, W, b)
    res = run_bass_kernel_spmd(nc, in_maps, core_ids=list(range(N_CORES)))

    full = np.empty((E_TOTAL, OUT_DIM), dtype=np.float32)
    for c in range(N_CORES):
        full[c * SHARD : (c + 1) * SHARD] = res.results[c]["out"][:SHARD]
    return full


def reference_numpy(bond_feat, bond_index, pos_nodes, W, b):
    """Pure-numpy oracle for local testing."""
    diff = pos_nodes[bond_index[0]] - pos_nodes[bond_index[1]]
    dist = np.sqrt(np.sum(diff * diff, axis=-1))
    offs_row = np.linspace(0.0, CUTOFF, NG, dtype=np.float32)
    dd = dist[:, None] - offs_row[None, :]
    gauss = np.exp(COEFF * dd * dd)
    feat = np.concatenate([bond_feat, gauss.astype(np.float32)], axis=-1)
    return feat @ W + b


# revision 23
# speedup vs baseline: 1.0198x; 1.0198x over previous
"""Trainium2 Bass kernel for BondEmbedding (GNN edge embedding).

out[e, :] = concat(bond_feat[e], gaussian_smearing(|pos[i0[e]] - pos[i1[e]]|)) @ W + b

Sharding: edges split across 8 NeuronCores (embarrassingly parallel);
pos table / weights / constants replicated on every core.

Per-core dataflow (supertile = 4096 edges, K=32 edges per SBUF partition,
edge e0 + p*K + kk lives at slot (p, kk)):
  - HWDGE DMA: bond_feat slab, block-idx / remainder slabs, output stores
  - SWDGE dma_gather (gpsimd queues 0/1, issued one supertile ahead): the
    pos table is packed [25000, 64] f32 (4 nodes per 256B block, each node
    a 16B row); per edge endpoint the 256B block is gathered (block index
    fits the required int16), then the node's 16B row is selected on DVE
    via a 4-wide one-hot and a grouped reduce.
  - DVE: one-hot select of each endpoint's 16B row; diff/square/reduce
    -> dist2, clamp; d = dist2 * rsqrt(dist2) via the int bit-hack seed
    + 2 Newton iterations (keeps ACT's activation-table fixed on the exp
    set -- the table-load pass picks disjoint sets for Ln and Exp,
    costing 2x1283ns per supertile otherwise); u = d - offsets, u^2
  - ACT: gauss = exp(coeff*u^2); psum -> sbuf bf16 copies of bond featT
  - PE (bf16 matmul = 1 cycle/row vs 4 for f32): bond features are
    transposed two 64-wide chunks at a time ([128,128] f32 -> [128,128]
    psum), gauss+ones four 21-wide chunks at a time ([128,84] f32 ->
    [84,128] psum); after bf16 copies to SBUF, each 128-edge chunk is
    two accumulating matmuls (K=64 bond + K=21 gauss) against the
    matching row-blocks of Wb [85,128] bf16. Row 84 of the gauss block
    is constant 1.0 and row 84 of Wb is the bias, so the bias needs no
    separate add.
  - Pool: indirect-gather dispatch; gauss psum -> sbuf bf16 copies
  - DVE: out psum -> sbuf copy, one 4-chunk store DMA per group

bf16 is used only for the matmul operands (bond features, gauss features,
weights); positions/distances stay f32. Max rel err vs the f32 reference
is ~1e-3, well inside the 2e-2 gate.
"""

import sys

sys.path.insert(0, "/opt/trn_rl_repo")

import numpy as np

E_TOTAL = 2_000_000
N_NODES = 100_000
IN_DIM = 64
OUT_DIM = 128
NG = 20
CUTOFF = 10.0
FEAT = IN_DIM + NG + 1  # 85 = 64 bond + 20 gauss + 1 ones (bias row)
GPAD = 32  # gauss slot stride (20 gauss + 1 one + 11 pad); 32 aligns packs to legal PE base partitions {0,32,64}

N_BLOCKS = N_NODES // 4  # 25000 blocks of 4 nodes (256B each)
N_CORES = 8
SHARD = E_TOTAL // N_CORES  # 250000
K = 32                      # edges per partition per supertile
S = 128 * K                 # 4096 edges per supertile
NT = 62                     # supertiles per core
E_PC = S * NT               # 253952 edges per core (wrap-around overlap)

_DELTA = CUTOFF / (NG - 1)
COEFF = -0.5 / (_DELTA * _DELTA)

_prog_cache = {}
WORK_BUFS = 4


def build_program(e_pc, nt, k, repeat=1):
    """Build the per-core Bass program (identical on all cores).

    repeat>1 re-runs the whole edge sweep that many times (same inputs and
    outputs) — used only for slope-based wall-clock timing.
    """
    from concourse import bacc, mybir, tile, bass
    from concourse.masks import make_identity

    f32 = mybir.dt.float32
    i16 = mybir.dt.int16
    bf16 = mybir.dt.bfloat16
    i32 = mybir.dt.int32
    ALU = mybir.AluOpType
    ACT = mybir.ActivationFunctionType

    s = 128 * k

    nc = bacc.Bacc("TRN2", target_bir_lowering=False, debug=False)

    bond = nc.dram_tensor("bond_feat", [e_pc, IN_DIM], f32, kind="ExternalInput")
    blk0 = nc.dram_tensor("blk0", [nt, 128, (128 * k) // 16], i16, kind="ExternalInput")
    blk1 = nc.dram_tensor("blk1", [nt, 128, (128 * k) // 16], i16, kind="ExternalInput")
    rem0 = nc.dram_tensor("rem0", [nt, 128, k], f32, kind="ExternalInput")
    rem1 = nc.dram_tensor("rem1", [nt, 128, k], f32, kind="ExternalInput")
    tab = nc.dram_tensor("tab", [N_BLOCKS, 64], f32, kind="ExternalInput")
    wb2 = nc.dram_tensor("wb2", [128, OUT_DIM], bf16, kind="ExternalInput")
    wg3 = nc.dram_tensor("wg3", [85, OUT_DIM], bf16, kind="ExternalInput")
    offs = nc.dram_tensor("offs", [128, NG], f32, kind="ExternalInput")
    out = nc.dram_tensor("out", [e_pc, OUT_DIM], f32, kind="ExternalOutput")

    with tile.TileContext(nc) as tc:
        with (
            tc.tile_pool(name="const", bufs=1) as cpool,
            tc.tile_pool(name="work", bufs=WORK_BUFS) as pool,
            tc.tile_pool(name="ft", bufs=8) as fpool,
            tc.tile_pool(name="gth", bufs=3) as gpool,
            tc.tile_pool(name="scratch", bufs=2) as spool,
            tc.tile_pool(name="psumb", bufs=3, space="PSUM") as ppoolb,
            tc.tile_pool(name="psumg", bufs=2, space="PSUM") as ppoolg,
            tc.tile_pool(name="psumo", bufs=3, space="PSUM") as ppoolo,
        ):
            wb2_sb = cpool.tile([128, OUT_DIM], bf16, tag="wb2")
            nc.sync.dma_start(out=wb2_sb[:], in_=wb2[:, :])
            wg3_sb = cpool.tile([85, OUT_DIM], bf16, tag="wg3")
            nc.sync.dma_start(out=wg3_sb[:], in_=wg3[:, :])
            offs_sb = cpool.tile([128, NG], f32, tag="offs")
            nc.sync.dma_start(out=offs_sb[:], in_=offs[:, :])
            ident = cpool.tile([128, 128], f32, tag="ident")
            make_identity(nc, ident[:])
            magic_sb = cpool.tile([128, 1], i32, tag="magic")
            nc.vector.memset(magic_sb[:], 0x5F3759DF)
            cand_sb = cpool.tile([128, 4], f32, tag="cand")
            nc.gpsimd.iota(cand_sb[:], pattern=[[1, 4]], base=0, channel_multiplier=0, allow_small_or_imprecise_dtypes=True)

            nw = s // 16

            def load_tiles(t):
                # issued two iterations ahead so the in-order SP queue
                # never parks loads behind store waits
                bt0 = pool.tile([128, nw], i16, tag="bt0")
                nc.sync.dma_start(out=bt0[:], in_=blk0[t, :, :])
                bt1 = pool.tile([128, nw], i16, tag="bt1")
                nc.sync.dma_start(out=bt1[:], in_=blk1[t, :, :])
                rt0 = pool.tile([128, k], f32, tag="rt0")
                nc.sync.dma_start(out=rt0[:], in_=rem0[t, :, :])
                rt1 = pool.tile([128, k], f32, tag="rt1")
                nc.sync.dma_start(out=rt1[:], in_=rem1[t, :, :])
                bf = pool.tile([128, IN_DIM * k], f32, tag="bf")
                nc.sync.dma_start(
                    out=bf[:],
                    in_=bond[t * s : t * s + s, :].rearrange(
                        "(p k) f -> p (k f)", p=128
                    ),
                )
                return (bt0, bt1, rt0, rt1), bf

            def gather(bts):
                # issued one iteration ahead: blocks are resident when the
                # iteration's DVE select/distance chain starts
                gths = []
                for r in range(2):
                    gth = gpool.tile([128, (s // 128) * 64], f32, tag=f"gth{r}")
                    nc.gpsimd.dma_gather(
                        out_ap=gth[:].rearrange("p (k c) -> p k c", c=64),
                        in_ap=tab[:, :],
                        idxs_ap=bts[r][:],
                        num_idxs=s,
                        num_idxs_reg=s,
                        elem_size=64,
                        single_packet=False,
                        queue_num=0,
                    )
                    gths.append(gth)
                return gths

            npack = (k + 2) // 3
            ngrp = (npack + 3) // 4

            cur = load_tiles(0)
            nxt = load_tiles(1 % nt)
            nxt2 = load_tiles(2 % nt)
            gth = gather(cur[0])
            gth_nxt = gather(nxt[0])
            for step in range(nt * repeat):
                t = step % nt
                e0 = t * s
                (bt0, bt1, rt0, rt1), bf = cur
                fut = load_tiles((t + 3) % nt)
                gth_nxt2 = gather(nxt2[0])
                cur, nxt, nxt2 = nxt, nxt2, fut

                # --- PE: bond transposes first (bf is already resident) --
                # 2 chunks per transpose ([128,128] f32); chunk kk lands at
                # partition base 64*(kk%2), col slot (kk//2)%4 of ptb kk//8.
                ptbs = []
                for j in range(k // 8):
                    ptb = ppoolb.tile([128, 4 * 128], f32, tag="ptb")
                    for i in range(4):
                        nc.tensor.transpose(
                            out=ptb[:, 128 * i : 128 * (i + 1)],
                            in_=bf[:, 128 * (4 * j + i) : 128 * (4 * j + i) + 128],
                            identity=ident[:],
                        )
                    ptbs.append(ptb)

                # --- DVE: select node rows, then distance chain ----------
                sel = []
                for r, rt in enumerate((rt0, rt1)):
                    oh = spool.tile([128, 4 * k], f32, tag=f"oh{r}")
                    nc.vector.tensor_tensor(
                        out=oh[:].rearrange("p (k m) -> p k m", m=4),
                        in0=rt[:].unsqueeze(2).to_broadcast([128, k, 4]),
                        in1=cand_sb[:].unsqueeze(1).to_broadcast([128, k, 4]),
                        op=ALU.is_equal,
                    )
                    tmp = spool.tile([128, 16 * k], f32, tag=f"tmp{r}")
                    gv = gth[r][:].rearrange("p (k m v) -> p k v m", m=4, v=16)
                    nc.vector.tensor_tensor(
                        out=tmp[:].rearrange("p (k c m) -> p k c m", c=4, m=4),
                        in0=gv[:, :, 0:4, :],
                        in1=oh[:]
                        .rearrange("p (k m) -> p k m", m=4)
                        .unsqueeze(2)
                        .to_broadcast([128, k, 4, 4]),
                        op=ALU.mult,
                    )
                    rr = spool.tile([128, 4 * k], f32, tag=f"r{r}")
                    nc.vector.tensor_reduce(
                        out=rr[:].rearrange("p (k c) -> p k c", c=4),
                        in_=tmp[:].rearrange("p (k c m) -> p k c m", c=4, m=4),
                        axis=mybir.AxisListType.X,
                        op=ALU.add,
                    )
                    sel.append(rr)
                diff = spool.tile([128, 4 * k], f32, tag="diff")
                diffv = diff[:].rearrange("p (k c) -> p k c", c=4)
                nc.vector.tensor_tensor(
                    out=diff[:], in0=sel[0][:], in1=sel[1][:], op=ALU.subtract
                )
                sq = spool.tile([128, 4 * k], f32, tag="sq")
                nc.vector.tensor_tensor(out=sq[:], in0=diff[:], in1=diff[:], op=ALU.mult)
                dist2 = spool.tile([128, k], f32, tag="dist2")
                nc.vector.tensor_reduce(
                    out=dist2[:],
                    in_=sq[:].rearrange("p (k c) -> p k c", c=4),
                    axis=mybir.AxisListType.X,
                    op=ALU.add,
                )
                # clamp keeps the Newton iterate finite (x*y^2 stays ~1)
                nc.vector.tensor_scalar_max(out=dist2[:], in0=dist2[:], scalar1=1e-30)
                ti = spool.tile([128, k], i32, tag="ti")
                nc.vector.tensor_scalar(
                    out=ti[:], in0=dist2[:].bitcast(i32), scalar1=1, scalar2=None,
                    op0=ALU.logical_shift_right,
                )
                y = spool.tile([128, k], f32, tag="y")
                nc.vector.tensor_tensor(
                    out=y[:].bitcast(i32),
                    in0=magic_sb[:].to_broadcast([128, k]),
                    in1=ti[:],
                    op=ALU.subtract,
                )
                nt1 = spool.tile([128, k], f32, tag="nt1")
                for _ in range(2):
                    nc.vector.tensor_tensor(out=nt1[:], in0=y[:], in1=y[:], op=ALU.mult)
                    # nt1 = (nt1 * -0.5) * dist2 ; y = (nt1 + 1.5) * y
                    nc.vector.scalar_tensor_tensor(
                        out=nt1[:], in0=nt1[:], scalar=-0.5, in1=dist2[:],
                        op0=ALU.mult, op1=ALU.mult,
                    )
                    nc.vector.scalar_tensor_tensor(
                        out=y[:], in0=nt1[:], scalar=1.5, in1=y[:],
                        op0=ALU.add, op1=ALU.mult,
                    )
                d = spool.tile([128, k], f32, tag="d")
                nc.vector.tensor_tensor(out=d[:], in0=dist2[:], in1=y[:], op=ALU.mult)

                # --- gauss u^2 in padded [128, k, GPAD] f32 tile ---------
                # slots NG:GPAD are zeroed: after the fused exp they become
                # 1.0 -- slot NG is the ones row that carries the bias.
                u = spool.tile([128, NG * k], f32, tag="u")
                uv = u[:].rearrange("p (k g) -> p k g", g=NG)
                nc.vector.tensor_tensor(
                    out=uv,
                    in0=d[:].unsqueeze(2).to_broadcast([128, k, NG]),
                    in1=offs_sb[:].unsqueeze(1).to_broadcast([128, k, NG]),
                    op=ALU.subtract,
                )
                uq = spool.tile([128, GPAD * k], f32, tag="uq")
                uqv = uq[:].rearrange("p (k g) -> p k g", g=GPAD)
                nc.vector.memset(uqv[:, :, NG:GPAD], 0.0)
                nc.vector.tensor_tensor(
                    out=uqv[:, :, 0:NG], in0=uv, in1=uv, op=ALU.mult
                )

                # --- PE: transpose u^2; ACT fuses exp into psum->sbuf ----
                # 3 chunks per transpose ([128,96] f32 incl pad); chunk kk
                # at base 32*(kk%3), col slot (kk//3)%4 of ftg (kk//12).
                gt3 = uq[:].rearrange("p (k g) -> p k g", g=GPAD)
                ftgs = []
                for g in range(ngrp):
                    pk0 = 4 * g
                    npk = min(4, npack - pk0)
                    ptg = ppoolg.tile([96, 4 * 128], f32, tag="ptg")
                    for pi in range(npk):
                        c0 = 3 * (pk0 + pi)
                        ncch = min(3, k - c0)
                        if ncch < 3:
                            nc.vector.memset(
                                ptg[32 * ncch : 96, 128 * pi : 128 * (pi + 1)], 0.0
                            )
                        nc.tensor.transpose(
                            out=ptg[0 : 32 * ncch, 128 * pi : 128 * (pi + 1)],
                            in_=gt3[:, c0 : c0 + ncch, :],
                            identity=ident[:],
                        )
                    ftg = fpool.tile([96, 4 * 128], bf16, tag="ftg")
                    nc.scalar.activation(
                        out=ftg[:, 0 : 128 * npk],
                        in_=ptg[:, 0 : 128 * npk],
                        func=ACT.Exp,
                        scale=COEFF,
                    )
                    ftgs.append(ftg)
                # DVE: bond psum -> sbuf bf16 (emitted after the distance
                # chain, so it never blocks the next iteration's DVE work)
                ftbs = []
                for j in range(k // 8):
                    ftb = fpool.tile([128, 4 * 128], bf16, tag="ftb")
                    nc.vector.tensor_copy(out=ftb[:], in_=ptbs[j][:])
                    ftbs.append(ftb)

                # --- matmuls + out copies + stores -----------------------
                ov = out[e0 : e0 + s, :].rearrange("(p kk) o -> p kk o", p=128)
                po = None
                for kk in range(k):
                    m = kk % 4
                    if m == 0:
                        po = ppoolo.tile([128, 4 * 128], f32, tag="po")
                    ftb = ftbs[kk // 8]
                    i = kk % 8
                    pk, gm = kk // 3, kk % 3
                    nc.tensor.matmul(
                        out=po[:, 128 * m : 128 * (m + 1)],
                        lhsT=ftb[
                            64 * (i % 2) : 64 * (i % 2) + 64,
                            128 * (i // 2) : 128 * (i // 2) + 128,
                        ],
                        rhs=wb2_sb[64 * (i % 2) : 64 * (i % 2) + 64, :],
                        start=True,
                        stop=False,
                    )
                    nc.tensor.matmul(
                        out=po[:, 128 * m : 128 * (m + 1)],
                        lhsT=ftgs[pk // 4][
                            32 * gm : 32 * gm + 21,
                            128 * (pk % 4) : 128 * (pk % 4) + 128,
                        ],
                        rhs=wg3_sb[32 * gm : 32 * gm + 21, :],
                        start=False,
                        stop=True,
                    )
                    if m == 3:
                        osb = fpool.tile([128, 4 * 128], f32, tag="osb")
                        nc.scalar.activation(out=osb[:], in_=po[:], func=ACT.Copy)
                        nc.sync.dma_start(
                            out=ov[:, kk - 3 : kk + 1, :],
                            in_=osb[:].rearrange("p (q o) -> p q o", o=OUT_DIM),
                        )

                gth, gth_nxt = gth_nxt, gth_nxt2

    nc.compile()
    return nc


def get_program(e_pc=E_PC, nt=NT, k=K):
    key = (e_pc, nt, k)
    if key not in _prog_cache:
        _prog_cache[key] = build_program(e_pc, nt, k)
    return _prog_cache[key]


def make_in_maps(bond_feat, bond_index, pos_nodes, W, b, e_pc=E_PC, nt=NT, k=K):
    """Shard the full problem into per-core input maps.

    Core c handles edges [c*SHARD, c*SHARD + e_pc) (wrapping around at
    E_TOTAL); rows beyond the first SHARD are redundant overlap so every
    core runs the identical static program.
    """
    import ml_dtypes

    bond_feat = np.ascontiguousarray(bond_feat, dtype=np.float32)
    idx0_all = np.ascontiguousarray(bond_index[0], dtype=np.int32)
    idx1_all = np.ascontiguousarray(bond_index[1], dtype=np.int32)

    tab = np.zeros((N_NODES, 16), dtype=np.float32)
    tab[:, :3] = pos_nodes
    tab = tab.reshape(N_BLOCKS, 64)

    offs_row = np.linspace(0.0, CUTOFF, NG, dtype=np.float32)
    offs_bcast = np.ascontiguousarray(np.broadcast_to(offs_row, (128, NG)))

    Wf = np.asarray(W, dtype=np.float32)
    wb2 = np.concatenate([Wf[:IN_DIM], Wf[:IN_DIM]], axis=0).astype(ml_dtypes.bfloat16)
    wg = np.concatenate(
        [Wf[IN_DIM:], np.asarray(b, dtype=np.float32)[None, :]], axis=0
    )  # [21, 128]
    wg3 = np.zeros((85, OUT_DIM), dtype=np.float32)
    for base in (0, 32, 64):
        wg3[base : base + 21] = wg
    wg3 = wg3.astype(ml_dtypes.bfloat16)

    def wrap_slice(arr, start, n):
        end = start + n
        if end <= E_TOTAL:
            return arr[start:end]
        return np.concatenate([arr[start:], arr[: end - E_TOTAL]], axis=0)

    in_maps = []
    for c in range(N_CORES):
        start = c * SHARD
        i0 = wrap_slice(idx0_all, start, e_pc)
        i1 = wrap_slice(idx1_all, start, e_pc)
        b0, r0 = _gather_inputs(i0, nt, k)
        b1, r1 = _gather_inputs(i1, nt, k)
        in_maps.append(
            {
                "bond_feat": wrap_slice(bond_feat, start, e_pc),
                "blk0": b0,
                "blk1": b1,
                "rem0": r0,
                "rem1": r1,
                "tab": tab,
                "wb2": wb2,
                "wg3": wg3,
                "offs": offs_bcast,
            }
        )
    return in_maps


def _gather_inputs(idx, nt, k):
    """blk (wrapped+replicated int16 block idx) and rem (f32 idx%4) slabs."""
    s = 128 * k
    nw = s // 16
    # gather-position i covers local edge (i%128)*k + i//128
    ii = np.arange(s)
    perm = (ii % 128) * k + (ii // 128)
    blk = (idx >> 2).astype(np.int16).reshape(nt, s)[:, perm]  # [nt, s]
    wrapped = blk.reshape(nt, nw, 16).transpose(0, 2, 1)  # [nt, 16, nw]
    blk_t = np.broadcast_to(wrapped[:, None, :, :], (nt, 8, 16, nw)).reshape(
        nt, 128, nw
    )
    rem = (idx & 3).astype(np.float32).reshape(nt, 128, k)
    return np.ascontiguousarray(blk_t), np.ascontiguousarray(rem)


def kernel(bond_feat, bond_index, pos_nodes, W, b):
    from concourse.bass_utils import run_bass_kernel_spmd

    nc = get_program()
    in_maps = make_in_maps(bond_feat, bond_index, pos_nodes, W, b)
    res = run_bass_kernel_spmd(nc, in_maps, core_ids=list(range(N_CORES)))

    full = np.empty((E_TOTAL, OUT_DIM), dtype=np.float32)
    for c in range(N_CORES):
        full[c * SHARD : (c + 1) * SHARD] = res.results[c]["out"][:SHARD]
    return full


def reference_numpy(bond_feat, bond_index, pos_nodes, W, b):
    """Pure-numpy oracle for local testing."""
    diff = pos_nodes[bond_index[0]] - pos_nodes[bond_index[1]]
    dist = np.sqrt(np.sum(diff * diff, axis=-1))
    offs_row = np.linspace(0.0, CUTOFF, NG, dtype=np.float32)
    dd = dist[:, None] - offs_row[None, :]
    gauss = np.exp(COEFF * dd * dd)
    feat = np.concatenate([bond_feat, gauss.astype(np.float32)], axis=-1)
    return feat @ W + b
